# revision 1
# baseline (speedup 1.0000x reference)
"""GroupPointNet kernel for 8 Trainium2 NeuronCores.

Strategy:
- FPS + KNN index selection run on host in jax-CPU with the exact reference
  op order (argmax/top-k tie-breaking must match the oracle bit-for-bit;
  a diverged FPS trajectory corrupts every downstream output position).
- The dense pipeline — 3x (1x1 conv matmul + LeakyReLU + BatchNorm) + max-pool
  over K — runs on the 8 cores, data-parallel over the B*M (b,m) groups,
  with in-kernel AllReduce for the global BatchNorm statistics.
"""

import numpy as np

SAMPLE_RATIO = 0.25
K = 20
SLOPE = 0.2
EPS = 1e-5

B, N, C = 4, 8192, 64
M = int(N * SAMPLE_RATIO)          # 2048
L = B * M * K                      # 163840 columns, ordered (b, m, k)
N_CORES = 8
GROUPS = B * M                     # 8192 (b,m) groups
GPC = GROUPS // N_CORES            # 1024 groups per core
LC = GPC * K                       # 20480 columns per core
# column chunks per core: multiples of K so max-pool groups never straddle
CHUNK = 500                        # 25 groups
CHUNKS = [(i * CHUNK, CHUNK) for i in range(LC // CHUNK)]
_rem = LC - (LC // CHUNK) * CHUNK
if _rem:
    CHUNKS.append(((LC // CHUNK) * CHUNK, _rem))
NCH = len(CHUNKS)

_CACHE = {}


def _host_indices(p_np):
    """FPS + KNN with reference-identical numerics on jax CPU."""
    import jax
    import jax.numpy as jnp
    from jax import lax

    cpu = jax.devices("cpu")[0]

    def fps(p, m):
        B_, N_, _ = p.shape

        def step(carry, _):
            dist, last_idx = carry
            last_pt = jnp.take_along_axis(p, last_idx[:, None, None], axis=1)
            d = jnp.sum((p - last_pt) ** 2, axis=-1)
            dist = jnp.minimum(dist, d)
            nxt = jnp.argmax(dist, axis=1).astype(jnp.int32)
            return (dist, nxt), last_idx

        dist0 = jnp.full((B_, N_), 1e10, dtype=p.dtype)
        idx0 = jnp.zeros((B_,), dtype=jnp.int32)
        _, idxs = lax.scan(step, (dist0, idx0), None, length=m)
        return jnp.transpose(idxs)

    def knn_idx(q, p, k):
        d = (jnp.sum(q * q, -1)[:, :, None]
             + jnp.sum(p * p, -1)[:, None, :]
             - 2.0 * jnp.einsum('bmd,bnd->bmn', q, p))
        _, idx = lax.top_k(-d, k)
        return idx

    with jax.default_device(cpu):
        p = jnp.asarray(p_np)
        idx = jax.jit(fps, static_argnums=1)(p, M)
        p1 = jnp.take_along_axis(p, idx[:, :, None], axis=1)
        nidx = jax.jit(knn_idx, static_argnums=2)(p1, p, K)
        return np.asarray(p1), np.asarray(nidx)


def _apply_drain_patch():
    """This walrus build rejects >1 sync wait on a CTRL-format instruction;
    split the TileContext kernel-tail drain's waits across single-wait NoOps."""
    import concourse.tile as tile_mod
    import concourse.mybir as mybir
    from concourse.vector_clock import ScopedClock

    def _split_drain_and_barrier(self, tick_clock, wait_clock):
        nc = self.nc
        drain_inst = nc.sync.drain()
        wait_clock.add_sem_waits(
            drain_inst.ins, ScopedClock({None: tick_clock.global_clock})
        )
        si = drain_inst.ins.sync_info
        if si is not None and si.on_wait and len(si.on_wait) > 1:
            waits = list(si.on_wait)
            si.on_wait = waits[:1]
            for w in waits[1:]:
                nop = nc.sync.nop(nofuse=True)
                nop.ins.sync_info = mybir.SyncInfo(on_wait=[w], on_update=[])
        nc.all_engine_barrier()
        assert self.sems is not None
        popped = nc._tile_sem_poison_stack.pop()
        assert popped is self._sem_poison
        nc.clear_and_free_semaphores(list(self.sems.allocated().values()))
        nc.all_engine_barrier()

    tile_mod.TileContext._drain_and_barrier = _split_drain_and_barrier


def _split_multi_waits(nc):
    """This walrus build allows only ONE sync wait per instruction (any
    format). Hoist extra waits onto same-engine NoOps inserted just before
    the owning instruction — in-order engines make this equivalent."""
    import concourse.mybir as mybir

    cnt = 0
    for f in nc.m.functions:
        for blk in f.blocks:
            changed = False
            out = []
            for ins in blk.instructions:
                si = ins.sync_info
                if si is not None and si.on_wait and len(si.on_wait) > 1:
                    waits = list(si.on_wait)
                    for w in waits[:-1]:
                        nop = mybir.InstNoOp(name=f"wsplit_{cnt}", ins=[], outs=[])
                        cnt += 1
                        nop.engine = ins.engine
                        nop.sync_info = mybir.SyncInfo(on_wait=[w], on_update=[])
                        out.append(nop)
                    si.on_wait = waits[-1:]
                    changed = True
                out.append(ins)
            if changed:
                blk.instructions = out
    return cnt


def _build_nc():
    import concourse.bass as bass
    import concourse.mybir as mybir
    import concourse.tile as tile

    _apply_drain_patch()
    dt = mybir.dt.float32
    Alu = mybir.AluOpType
    Act = mybir.ActivationFunctionType

    nc = bass.Bass("TRN2", target_bir_lowering=False, debug=False,
                   num_devices=N_CORES)

    xc = nc.dram_tensor("xc", [6, LC], dt, kind="ExternalInput")
    w1t = nc.dram_tensor("w1t", [6, C], dt, kind="ExternalInput")
    w2t = nc.dram_tensor("w2t", [C, C], dt, kind="ExternalInput")
    w3t = nc.dram_tensor("w3t", [C, C], dt, kind="ExternalInput")
    gb = nc.dram_tensor("gb", [C, 6], dt, kind="ExternalInput")
    y = nc.dram_tensor("y", [C, GPC], dt, kind="ExternalOutput")

    inv_count = 1.0 / float(L)

    with tile.TileContext(nc) as tc:
        with (
            tc.tile_pool(name="const", bufs=1) as cpool,
            tc.tile_pool(name="slab", bufs=1) as slab,
            tc.tile_pool(name="chunk", bufs=3) as ch,
            tc.tile_pool(name="psum", bufs=4, space="PSUM") as pp,
            tc.tile_pool(name="stats", bufs=1) as sp,
            tc.tile_pool(name="dram", bufs=1, space="DRAM") as dram,
        ):
            w1s = cpool.tile([6, C], dt, tag="w1")
            w2s = cpool.tile([C, C], dt, tag="w2")
            w3s = cpool.tile([C, C], dt, tag="w3")
            gbs = cpool.tile([C, 6], dt, tag="gb")
            nc.sync.dma_start(w1s[:], w1t[:])
            nc.sync.dma_start(w2s[:], w2t[:])
            nc.sync.dma_start(w3s[:], w3t[:])
            nc.sync.dma_start(gbs[:], gb[:])

            z1 = slab.tile([C, LC], dt, tag="slabA")
            z2 = slab.tile([C, LC], dt, tag="slabB")

            ssum = sp.tile([C, NCH], dt, tag="ssum1")
            qsum = sp.tile([C, NCH], dt, tag="qsum1")

            def stats_and_scale(layer, s_tile, q_tile, g_col, b_col):
                """Reduce per-chunk stats, AllReduce across cores, produce
                per-channel (scale, bias) implementing BN."""
                st = sp.tile([C, 2], dt, tag=f"st{layer}")
                nc.vector.tensor_reduce(st[:, 0:1], s_tile[:, :NCH],
                                        mybir.AxisListType.X, Alu.add)
                nc.vector.tensor_reduce(st[:, 1:2], q_tile[:, :NCH],
                                        mybir.AxisListType.X, Alu.add)
                cc_in = dram.tile([C, 2], dt, tag=f"ccin{layer}")
                cc_out = dram.tile([C, 2], dt, tag=f"ccout{layer}")
                nc.sync.dma_start(cc_in[:], st[:])
                nc.gpsimd.collective_compute(
                    "AllReduce", Alu.add,
                    replica_groups=[list(range(N_CORES))],
                    ins=[cc_in[:]], outs=[cc_out[:]],
                )
                gst = sp.tile([C, 2], dt, tag=f"gst{layer}")
                nc.sync.dma_start(gst[:], cc_out[:])
                mean = sp.tile([C, 1], dt, tag=f"mean{layer}")
                ex2 = sp.tile([C, 1], dt, tag=f"ex2{layer}")
                var = sp.tile([C, 1], dt, tag=f"var{layer}")
                sd = sp.tile([C, 1], dt, tag=f"sd{layer}")
                inv = sp.tile([C, 1], dt, tag=f"inv{layer}")
                scale = sp.tile([C, 1], dt, tag=f"scale{layer}")
                bias = sp.tile([C, 1], dt, tag=f"bias{layer}")
                nc.vector.tensor_scalar_mul(mean[:], gst[:, 0:1], inv_count)
                nc.vector.tensor_scalar_mul(ex2[:], gst[:, 1:2], inv_count)
                nc.vector.tensor_mul(var[:], mean[:], mean[:])
                nc.vector.tensor_sub(var[:], ex2[:], var[:])
                nc.vector.tensor_scalar_add(var[:], var[:], EPS)
                nc.scalar.activation(sd[:], var[:], Act.Sqrt, bias=0.0)
                nc.vector.reciprocal(inv[:], sd[:])
                nc.vector.tensor_mul(scale[:], g_col, inv[:])
                nc.vector.tensor_mul(bias[:], mean[:], scale[:])
                nc.vector.tensor_sub(bias[:], b_col, bias[:])
                return scale, bias

            # ---- layer 1: conv1 + leaky + stats (input streamed from DRAM)
            for i, (off, w) in enumerate(CHUNKS):
                xt = ch.tile([6, CHUNK], dt, tag="xin")
                nc.sync.dma_start(xt[:, :w], xc[:, off:off + w])
                ps = pp.tile([C, CHUNK], dt, tag="ps")
                nc.tensor.matmul(ps[:, :w], w1s[:], xt[:, :w],
                                 start=True, stop=True)
                zr = ch.tile([C, CHUNK], dt, tag="zraw")
                nc.scalar.activation(zr[:, :w], ps[:, :w], Act.Copy, bias=0.0)
                nc.vector.scalar_tensor_tensor(
                    z1[:, off:off + w], zr[:, :w], SLOPE, zr[:, :w],
                    Alu.mult, Alu.max, accum_out=ssum[:, i:i + 1])
                scr = ch.tile([C, CHUNK], dt, tag="scr")
                nc.scalar.activation(scr[:, :w], z1[:, off:off + w], Act.Square,
                                     accum_out=qsum[:, i:i + 1])

            sc1, bi1 = stats_and_scale(1, ssum, qsum, gbs[:, 0:1], gbs[:, 1:2])

            ssum2 = sp.tile([C, NCH], dt, tag="ssum2")
            qsum2 = sp.tile([C, NCH], dt, tag="qsum2")

            # ---- layer 2: BN1-apply + conv2 + leaky + stats
            for i, (off, w) in enumerate(CHUNKS):
                xt = ch.tile([C, CHUNK], dt, tag="xbn")
                nc.vector.tensor_scalar(xt[:, :w], z1[:, off:off + w],
                                        sc1[:], bi1[:], Alu.mult, Alu.add)
                ps = pp.tile([C, CHUNK], dt, tag="ps")
                nc.tensor.matmul(ps[:, :w], w2s[:], xt[:, :w],
                                 start=True, stop=True)
                zr = ch.tile([C, CHUNK], dt, tag="zraw")
                nc.scalar.activation(zr[:, :w], ps[:, :w], Act.Copy, bias=0.0)
                nc.vector.scalar_tensor_tensor(
                    z2[:, off:off + w], zr[:, :w], SLOPE, zr[:, :w],
                    Alu.mult, Alu.max, accum_out=ssum2[:, i:i + 1])
                scr = ch.tile([C, CHUNK], dt, tag="scr")
                nc.scalar.activation(scr[:, :w], z2[:, off:off + w], Act.Square,
                                     accum_out=qsum2[:, i:i + 1])

            sc2, bi2 = stats_and_scale(2, ssum2, qsum2, gbs[:, 2:3], gbs[:, 3:4])

            ssum3 = sp.tile([C, NCH], dt, tag="ssum3")
            qsum3 = sp.tile([C, NCH], dt, tag="qsum3")
            z3 = slab.tile([C, LC], dt, tag="slabA")  # reuse z1's slot

            # ---- layer 3: BN2-apply + conv3 + leaky + stats
            for i, (off, w) in enumerate(CHUNKS):
                xt = ch.tile([C, CHUNK], dt, tag="xbn")
                nc.vector.tensor_scalar(xt[:, :w], z2[:, off:off + w],
                                        sc2[:], bi2[:], Alu.mult, Alu.add)
                ps = pp.tile([C, CHUNK], dt, tag="ps")
                nc.tensor.matmul(ps[:, :w], w3s[:], xt[:, :w],
                                 start=True, stop=True)
                zr = ch.tile([C, CHUNK], dt, tag="zraw")
                nc.scalar.activation(zr[:, :w], ps[:, :w], Act.Copy, bias=0.0)
                nc.vector.scalar_tensor_tensor(
                    z3[:, off:off + w], zr[:, :w], SLOPE, zr[:, :w],
                    Alu.mult, Alu.max, accum_out=ssum3[:, i:i + 1])
                scr = ch.tile([C, CHUNK], dt, tag="scr")
                nc.scalar.activation(scr[:, :w], z3[:, off:off + w], Act.Square,
                                     accum_out=qsum3[:, i:i + 1])

            sc3, bi3 = stats_and_scale(3, ssum3, qsum3, gbs[:, 4:5], gbs[:, 5:6])

            # ---- BN3-apply + max-pool over K
            yslab = sp.tile([C, GPC], dt, tag="yslab")
            for i, (off, w) in enumerate(CHUNKS):
                yt = ch.tile([C, CHUNK], dt, tag="ybn")
                nc.vector.tensor_scalar(yt[:, :w], z3[:, off:off + w],
                                        sc3[:], bi3[:], Alu.mult, Alu.add)
                g0, ng = off // K, w // K
                nc.vector.tensor_reduce(
                    yslab[:, g0:g0 + ng],
                    yt[:, :w].rearrange("p (g k) -> p g k", k=K),
                    mybir.AxisListType.X, Alu.max)
            nc.sync.dma_start(y[:], yslab[:])

    _split_multi_waits(nc)
    return nc


def kernel(p, W1, g1, b1, W2, g2, b2, W3, g3, b3):
    from concourse import bass_utils

    p = np.asarray(p, np.float32)
    p1, nidx = _host_indices(p)

    batch = np.arange(B)[:, None, None]
    grouped = p[batch, nidx]                       # [B,M,K,3]
    dp = grouped - p1[:, :, None, :]               # [B,M,K,3]
    gf = np.concatenate([dp, grouped], axis=-1)    # [B,M,K,6]
    x = np.ascontiguousarray(
        gf.reshape(L, 6).T.astype(np.float32))     # [6, L], cols (b,m,k)

    if "nc" not in _CACHE:
        _CACHE["nc"] = _build_nc()
    nc = _CACHE["nc"]

    w1t = np.ascontiguousarray(np.asarray(W1, np.float32).T)  # [6,64]
    w2t = np.ascontiguousarray(np.asarray(W2, np.float32).T)  # [64,64]
    w3t = np.ascontiguousarray(np.asarray(W3, np.float32).T)
    gbm = np.stack([g1, b1, g2, b2, g3, b3], axis=1).astype(np.float32)

    in_maps = []
    for c in range(N_CORES):
        in_maps.append({
            "xc": np.ascontiguousarray(x[:, c * LC:(c + 1) * LC]),
            "w1t": w1t, "w2t": w2t, "w3t": w3t, "gb": gbm,
        })

    res = bass_utils.run_bass_kernel_spmd(nc, in_maps,
                                          core_ids=list(range(N_CORES)))
    ys = [res.results[c]["y"] for c in range(N_CORES)]     # each [64, 1024]
    Y = np.concatenate(ys, axis=1)                         # [64, 8192]
    out = Y.reshape(C, B, M).transpose(1, 0, 2)            # [B, 64, M]
    return np.ascontiguousarray(out.astype(np.float32))



# revision 2
# speedup vs baseline: 1.7160x; 1.7160x over previous
"""GroupPointNet kernel for 8 Trainium2 NeuronCores.

Strategy:
- FPS runs on host via numba (bit-exact with the jax-CPU reference: same IEEE
  op order, first-occurrence argmax; verified idx-identical). KNN top-20 runs
  on host jax-CPU (order differences at the k-boundary are set-invariant
  downstream through the max-pool, impact ~4e-4 RMS).
- The dense pipeline — 3x (1x1 conv matmul + LeakyReLU + BatchNorm) + max-pool
  over K — runs on the 8 cores, data-parallel over the B*M (b,m) groups,
  with in-kernel AllReduce for the global BatchNorm statistics.
- The PJRT executable is AOT-compiled once and cached; per-call work is just
  host index selection + input packing + one device dispatch.
"""

import numpy as np

SAMPLE_RATIO = 0.25
K = 20
SLOPE = 0.2
EPS = 1e-5

B, N, C = 4, 8192, 64
M = int(N * SAMPLE_RATIO)          # 2048
L = B * M * K                      # 163840 columns, ordered (b, m, k)
N_CORES = 8
GROUPS = B * M                     # 8192 (b,m) groups
GPC = GROUPS // N_CORES            # 1024 groups per core
LC = GPC * K                       # 20480 columns per core
# column chunks per core: multiples of K so max-pool groups never straddle
CHUNK = 500                        # 25 groups
CHUNKS = [(i * CHUNK, CHUNK) for i in range(LC // CHUNK)]
_rem = LC - (LC // CHUNK) * CHUNK
if _rem:
    CHUNKS.append(((LC // CHUNK) * CHUNK, _rem))
NCH = len(CHUNKS)

_CACHE = {}


# ---------------------------------------------------------------- host FPS
def _fps_numba():
    import numba

    @numba.njit(cache=True, fastmath=False)
    def fps_nb(p, m):
        B_, N_, _ = p.shape
        out = np.empty((B_, m), np.int32)
        dist = np.empty(N_, np.float32)
        for b in range(B_):
            for n in range(N_):
                dist[n] = 1e10
            last = 0
            for i in range(m):
                out[b, i] = last
                lx = p[b, last, 0]
                ly = p[b, last, 1]
                lz = p[b, last, 2]
                best = np.float32(-1.0)
                besti = 0
                for n in range(N_):
                    dx = p[b, n, 0] - lx
                    dy = p[b, n, 1] - ly
                    dz = p[b, n, 2] - lz
                    d = dx * dx + dy * dy
                    d = d + dz * dz
                    dn = dist[n]
                    if d < dn:
                        dn = d
                        dist[n] = dn
                    if dn > best:
                        best = dn
                        besti = n
                last = besti
        return out

    return fps_nb


def _fps_numpy(p, m):
    """Bit-exact numpy mirror of the reference FPS (same IEEE op order)."""
    B_, N_, _ = p.shape
    px, py, pz = p[..., 0], p[..., 1], p[..., 2]
    dist = np.full((B_, N_), 1e10, dtype=np.float32)
    last = np.zeros((B_,), dtype=np.int32)
    out = np.empty((m, B_), dtype=np.int32)
    ar = np.arange(B_)
    dx = np.empty_like(px)
    dy = np.empty_like(py)
    dz = np.empty_like(pz)
    for i in range(m):
        out[i] = last
        lp = p[ar, last]
        np.subtract(px, lp[:, 0:1], out=dx)
        np.multiply(dx, dx, out=dx)
        np.subtract(py, lp[:, 1:2], out=dy)
        np.multiply(dy, dy, out=dy)
        np.subtract(pz, lp[:, 2:3], out=dz)
        np.multiply(dz, dz, out=dz)
        np.add(dx, dy, out=dx)
        np.add(dx, dz, out=dx)
        np.minimum(dist, dx, out=dist)
        last = dist.argmax(axis=1).astype(np.int32)
    return np.ascontiguousarray(out.T)


def _fps(p):
    if "fps" not in _CACHE:
        try:
            _CACHE["fps"] = _fps_numba()
        except Exception:
            _CACHE["fps"] = _fps_numpy
    return np.asarray(_CACHE["fps"](p, M))


# ---------------------------------------------------------------- host KNN
def _knn(p1, p):
    import jax
    import jax.numpy as jnp
    from jax import lax

    if "knn" not in _CACHE:
        cpu = jax.devices("cpu")[0]

        def knn_idx(q, pp):
            d = (jnp.sum(q * q, -1)[:, :, None]
                 + jnp.sum(pp * pp, -1)[:, None, :]
                 - 2.0 * jnp.einsum('bmd,bnd->bmn', q, pp))
            _, idx = lax.top_k(-d, K)
            return idx

        _CACHE["knn"] = jax.jit(knn_idx, device=cpu)
    return np.asarray(_CACHE["knn"](p1, p))


# ---------------------------------------------------------------- bass build
def _apply_drain_patch():
    """This walrus build rejects >1 sync wait on a CTRL-format instruction;
    split the TileContext kernel-tail drain's waits across single-wait NoOps."""
    import concourse.tile as tile_mod
    import concourse.mybir as mybir
    from concourse.vector_clock import ScopedClock

    def _split_drain_and_barrier(self, tick_clock, wait_clock):
        nc = self.nc
        drain_inst = nc.sync.drain()
        wait_clock.add_sem_waits(
            drain_inst.ins, ScopedClock({None: tick_clock.global_clock})
        )
        si = drain_inst.ins.sync_info
        if si is not None and si.on_wait and len(si.on_wait) > 1:
            waits = list(si.on_wait)
            si.on_wait = waits[:1]
            for w in waits[1:]:
                nop = nc.sync.nop(nofuse=True)
                nop.ins.sync_info = mybir.SyncInfo(on_wait=[w], on_update=[])
        nc.all_engine_barrier()
        assert self.sems is not None
        popped = nc._tile_sem_poison_stack.pop()
        assert popped is self._sem_poison
        nc.clear_and_free_semaphores(list(self.sems.allocated().values()))
        nc.all_engine_barrier()

    tile_mod.TileContext._drain_and_barrier = _split_drain_and_barrier


def _split_multi_waits(nc):
    """This walrus build allows only ONE sync wait per instruction (any
    format). Hoist extra waits onto same-engine NoOps inserted just before
    the owning instruction — in-order engines make this equivalent."""
    import concourse.mybir as mybir

    cnt = 0
    for f in nc.m.functions:
        for blk in f.blocks:
            changed = False
            out = []
            for ins in blk.instructions:
                si = ins.sync_info
                if si is not None and si.on_wait and len(si.on_wait) > 1:
                    waits = list(si.on_wait)
                    for w in waits[:-1]:
                        nop = mybir.InstNoOp(name=f"wsplit_{cnt}", ins=[], outs=[])
                        cnt += 1
                        nop.engine = ins.engine
                        nop.sync_info = mybir.SyncInfo(on_wait=[w], on_update=[])
                        out.append(nop)
                    si.on_wait = waits[-1:]
                    changed = True
                out.append(ins)
            if changed:
                blk.instructions = out
    return cnt


def _build_nc():
    import concourse.bass as bass
    import concourse.mybir as mybir
    import concourse.tile as tile

    _apply_drain_patch()
    dt = mybir.dt.float32
    Alu = mybir.AluOpType
    Act = mybir.ActivationFunctionType

    nc = bass.Bass("TRN2", target_bir_lowering=False, debug=False,
                   num_devices=N_CORES)

    xc = nc.dram_tensor("xc", [6, LC], dt, kind="ExternalInput")
    w1t = nc.dram_tensor("w1t", [6, C], dt, kind="ExternalInput")
    w2t = nc.dram_tensor("w2t", [C, C], dt, kind="ExternalInput")
    w3t = nc.dram_tensor("w3t", [C, C], dt, kind="ExternalInput")
    gb = nc.dram_tensor("gb", [C, 6], dt, kind="ExternalInput")
    y = nc.dram_tensor("y", [C, GPC], dt, kind="ExternalOutput")

    inv_count = 1.0 / float(L)

    with tile.TileContext(nc) as tc:
        with (
            tc.tile_pool(name="const", bufs=1) as cpool,
            tc.tile_pool(name="slab", bufs=1) as slab,
            tc.tile_pool(name="chunk", bufs=3) as ch,
            tc.tile_pool(name="psum", bufs=4, space="PSUM") as pp,
            tc.tile_pool(name="stats", bufs=1) as sp,
            tc.tile_pool(name="dram", bufs=1, space="DRAM") as dram,
        ):
            w1s = cpool.tile([6, C], dt, tag="w1")
            w2s = cpool.tile([C, C], dt, tag="w2")
            w3s = cpool.tile([C, C], dt, tag="w3")
            gbs = cpool.tile([C, 6], dt, tag="gb")
            nc.sync.dma_start(w1s[:], w1t[:])
            nc.sync.dma_start(w2s[:], w2t[:])
            nc.sync.dma_start(w3s[:], w3t[:])
            nc.sync.dma_start(gbs[:], gb[:])

            z1 = slab.tile([C, LC], dt, tag="slabA")
            z2 = slab.tile([C, LC], dt, tag="slabB")

            ssum = sp.tile([C, NCH], dt, tag="ssum1")
            qsum = sp.tile([C, NCH], dt, tag="qsum1")

            def stats_and_scale(layer, s_tile, q_tile, g_col, b_col):
                """Reduce per-chunk stats, AllReduce across cores, produce
                per-channel (scale, bias) implementing BN."""
                st = sp.tile([C, 2], dt, tag=f"st{layer}")
                nc.vector.tensor_reduce(st[:, 0:1], s_tile[:, :NCH],
                                        mybir.AxisListType.X, Alu.add)
                nc.vector.tensor_reduce(st[:, 1:2], q_tile[:, :NCH],
                                        mybir.AxisListType.X, Alu.add)
                cc_in = dram.tile([C, 2], dt, tag=f"ccin{layer}")
                cc_out = dram.tile([C, 2], dt, tag=f"ccout{layer}")
                nc.sync.dma_start(cc_in[:], st[:])
                nc.gpsimd.collective_compute(
                    "AllReduce", Alu.add,
                    replica_groups=[list(range(N_CORES))],
                    ins=[cc_in[:]], outs=[cc_out[:]],
                )
                gst = sp.tile([C, 2], dt, tag=f"gst{layer}")
                nc.sync.dma_start(gst[:], cc_out[:])
                mean = sp.tile([C, 1], dt, tag=f"mean{layer}")
                ex2 = sp.tile([C, 1], dt, tag=f"ex2{layer}")
                var = sp.tile([C, 1], dt, tag=f"var{layer}")
                sd = sp.tile([C, 1], dt, tag=f"sd{layer}")
                inv = sp.tile([C, 1], dt, tag=f"inv{layer}")
                scale = sp.tile([C, 1], dt, tag=f"scale{layer}")
                bias = sp.tile([C, 1], dt, tag=f"bias{layer}")
                nc.vector.tensor_scalar_mul(mean[:], gst[:, 0:1], inv_count)
                nc.vector.tensor_scalar_mul(ex2[:], gst[:, 1:2], inv_count)
                nc.vector.tensor_mul(var[:], mean[:], mean[:])
                nc.vector.tensor_sub(var[:], ex2[:], var[:])
                nc.vector.tensor_scalar_add(var[:], var[:], EPS)
                nc.scalar.activation(sd[:], var[:], Act.Sqrt, bias=0.0)
                nc.vector.reciprocal(inv[:], sd[:])
                nc.vector.tensor_mul(scale[:], g_col, inv[:])
                nc.vector.tensor_mul(bias[:], mean[:], scale[:])
                nc.vector.tensor_sub(bias[:], b_col, bias[:])
                return scale, bias

            # ---- layer 1: conv1 + leaky + stats (input streamed from DRAM)
            for i, (off, w) in enumerate(CHUNKS):
                xt = ch.tile([6, CHUNK], dt, tag="xin")
                nc.sync.dma_start(xt[:, :w], xc[:, off:off + w])
                ps = pp.tile([C, CHUNK], dt, tag="ps")
                nc.tensor.matmul(ps[:, :w], w1s[:], xt[:, :w],
                                 start=True, stop=True)
                zr = ch.tile([C, CHUNK], dt, tag="zraw")
                nc.scalar.activation(zr[:, :w], ps[:, :w], Act.Copy, bias=0.0)
                nc.vector.scalar_tensor_tensor(
                    z1[:, off:off + w], zr[:, :w], SLOPE, zr[:, :w],
                    Alu.mult, Alu.max, accum_out=ssum[:, i:i + 1])
                scr = ch.tile([C, CHUNK], dt, tag="scr")
                nc.scalar.activation(scr[:, :w], z1[:, off:off + w], Act.Square,
                                     accum_out=qsum[:, i:i + 1])

            sc1, bi1 = stats_and_scale(1, ssum, qsum, gbs[:, 0:1], gbs[:, 1:2])

            ssum2 = sp.tile([C, NCH], dt, tag="ssum2")
            qsum2 = sp.tile([C, NCH], dt, tag="qsum2")

            # ---- layer 2: BN1-apply + conv2 + leaky + stats
            for i, (off, w) in enumerate(CHUNKS):
                xt = ch.tile([C, CHUNK], dt, tag="xbn")
                nc.vector.tensor_scalar(xt[:, :w], z1[:, off:off + w],
                                        sc1[:], bi1[:], Alu.mult, Alu.add)
                ps = pp.tile([C, CHUNK], dt, tag="ps")
                nc.tensor.matmul(ps[:, :w], w2s[:], xt[:, :w],
                                 start=True, stop=True)
                zr = ch.tile([C, CHUNK], dt, tag="zraw")
                nc.scalar.activation(zr[:, :w], ps[:, :w], Act.Copy, bias=0.0)
                nc.vector.scalar_tensor_tensor(
                    z2[:, off:off + w], zr[:, :w], SLOPE, zr[:, :w],
                    Alu.mult, Alu.max, accum_out=ssum2[:, i:i + 1])
                scr = ch.tile([C, CHUNK], dt, tag="scr")
                nc.scalar.activation(scr[:, :w], z2[:, off:off + w], Act.Square,
                                     accum_out=qsum2[:, i:i + 1])

            sc2, bi2 = stats_and_scale(2, ssum2, qsum2, gbs[:, 2:3], gbs[:, 3:4])

            ssum3 = sp.tile([C, NCH], dt, tag="ssum3")
            qsum3 = sp.tile([C, NCH], dt, tag="qsum3")
            z3 = slab.tile([C, LC], dt, tag="slabA")  # reuse z1's slot

            # ---- layer 3: BN2-apply + conv3 + leaky + stats
            for i, (off, w) in enumerate(CHUNKS):
                xt = ch.tile([C, CHUNK], dt, tag="xbn")
                nc.vector.tensor_scalar(xt[:, :w], z2[:, off:off + w],
                                        sc2[:], bi2[:], Alu.mult, Alu.add)
                ps = pp.tile([C, CHUNK], dt, tag="ps")
                nc.tensor.matmul(ps[:, :w], w3s[:], xt[:, :w],
                                 start=True, stop=True)
                zr = ch.tile([C, CHUNK], dt, tag="zraw")
                nc.scalar.activation(zr[:, :w], ps[:, :w], Act.Copy, bias=0.0)
                nc.vector.scalar_tensor_tensor(
                    z3[:, off:off + w], zr[:, :w], SLOPE, zr[:, :w],
                    Alu.mult, Alu.max, accum_out=ssum3[:, i:i + 1])
                scr = ch.tile([C, CHUNK], dt, tag="scr")
                nc.scalar.activation(scr[:, :w], z3[:, off:off + w], Act.Square,
                                     accum_out=qsum3[:, i:i + 1])

            sc3, bi3 = stats_and_scale(3, ssum3, qsum3, gbs[:, 4:5], gbs[:, 5:6])

            # ---- BN3-apply + max-pool over K
            yslab = sp.tile([C, GPC], dt, tag="yslab")
            for i, (off, w) in enumerate(CHUNKS):
                yt = ch.tile([C, CHUNK], dt, tag="ybn")
                nc.vector.tensor_scalar(yt[:, :w], z3[:, off:off + w],
                                        sc3[:], bi3[:], Alu.mult, Alu.add)
                g0, ng = off // K, w // K
                nc.vector.tensor_reduce(
                    yslab[:, g0:g0 + ng],
                    yt[:, :w].rearrange("p (g k) -> p g k", k=K),
                    mybir.AxisListType.X, Alu.max)
            nc.sync.dma_start(y[:], yslab[:])

    _split_multi_waits(nc)
    return nc


# ------------------------------------------------------- cached AOT executable
def _get_exec():
    """Build the bass module and AOT-compile the 8-core PJRT executable once.

    Returns (compiled, in_names, out_avals); call with concatenated per-core
    inputs followed by donated zero output buffers.
    """
    if "exec" in _CACHE:
        return _CACHE["exec"]

    import jax
    from concourse import bass2jax
    import concourse.mybir as mybir
    from jax.experimental.shard_map import shard_map
    from jax.sharding import Mesh, PartitionSpec

    nc = _build_nc()
    bass2jax.install_neuronx_cc_hook()

    partition_name = (nc.partition_id_tensor.name
                      if nc.partition_id_tensor else None)
    in_names, out_names, out_avals, zero_shapes = [], [], [], []
    in_shapes = {}
    for alloc in nc.m.functions[0].allocations:
        if not isinstance(alloc, mybir.MemoryLocationSet):
            continue
        name = alloc.memorylocations[0].name
        if alloc.kind == "ExternalInput":
            if name != partition_name:
                in_names.append(name)
                in_shapes[name] = (tuple(alloc.tensor_shape),
                                  mybir.dt.np(alloc.dtype))
        elif alloc.kind == "ExternalOutput":
            out_names.append(name)
            shape = tuple(alloc.tensor_shape)
            dtype = mybir.dt.np(alloc.dtype)
            out_avals.append(jax.core.ShapedArray(shape, dtype))
            zero_shapes.append((shape, dtype))
    n_params = len(in_names)
    n_outs = len(out_avals)
    all_in_names = list(in_names) + list(out_names)
    if partition_name is not None:
        all_in_names.append(partition_name)
    donate = tuple(range(n_params, n_params + n_outs))

    def _body(*args):
        operands = list(args)
        if partition_name is not None:
            operands.append(bass2jax.partition_id_tensor())
        outs = bass2jax._bass_exec_p.bind(
            *operands,
            out_avals=tuple(out_avals),
            in_names=tuple(all_in_names),
            out_names=tuple(out_names),
            lowering_input_output_aliases=(),
            sim_require_finite=True,
            sim_require_nnan=True,
            nc=nc,
        )
        return tuple(outs)

    devices = jax.devices()[:N_CORES]
    mesh = Mesh(np.asarray(devices), ("core",))
    in_specs = (PartitionSpec("core"),) * (n_params + n_outs)
    out_specs = (PartitionSpec("core"),) * n_outs

    def compile_fn():
        jitted = jax.jit(
            shard_map(_body, mesh=mesh, in_specs=in_specs,
                      out_specs=out_specs, check_rep=False),
            donate_argnums=donate, keep_unused=True)
        arg_specs = [
            jax.ShapeDtypeStruct((N_CORES * in_shapes[nm][0][0],
                                  *in_shapes[nm][0][1:]), in_shapes[nm][1])
            for nm in in_names
        ] + [
            jax.ShapeDtypeStruct((N_CORES * s[0], *s[1:]), d)
            for (s, d) in zero_shapes
        ]
        return jitted.lower(*arg_specs).compile()

    compiled = bass2jax.fast_dispatch_compile(compile_fn)
    _CACHE["exec"] = (compiled, in_names, zero_shapes, out_avals)
    return _CACHE["exec"]


def kernel(p, W1, g1, b1, W2, g2, b2, W3, g3, b3):
    p = np.ascontiguousarray(np.asarray(p, np.float32))
    idx = _fps(p)                                   # [B, M] int32
    p1 = np.take_along_axis(p, idx[:, :, None], axis=1)   # [B, M, 3]
    nidx = _knn(p1, p)                              # [B, M, K] int32

    batch = np.arange(B)[:, None, None]
    grouped = p[batch, nidx]                       # [B,M,K,3]
    dp = grouped - p1[:, :, None, :]               # [B,M,K,3]
    gf = np.concatenate([dp, grouped], axis=-1)    # [B,M,K,6]
    x = np.ascontiguousarray(
        gf.reshape(L, 6).T.astype(np.float32))     # [6, L], cols (b,m,k)

    compiled, in_names, zero_shapes, out_avals = _get_exec()

    w1t = np.ascontiguousarray(np.asarray(W1, np.float32).T)  # [6,64]
    w2t = np.ascontiguousarray(np.asarray(W2, np.float32).T)  # [64,64]
    w3t = np.ascontiguousarray(np.asarray(W3, np.float32).T)
    gbm = np.stack([g1, b1, g2, b2, g3, b3], axis=1).astype(np.float32)

    per_core = {
        "xc": [np.ascontiguousarray(x[:, c * LC:(c + 1) * LC])
               for c in range(N_CORES)],
    }
    rep = {"w1t": w1t, "w2t": w2t, "w3t": w3t, "gb": gbm}
    concat_in = []
    for nm in in_names:
        if nm in per_core:
            concat_in.append(np.concatenate(per_core[nm], axis=0))
        else:
            concat_in.append(np.concatenate([rep[nm]] * N_CORES, axis=0))
    czeros = [np.zeros((N_CORES * s[0], *s[1:]), d) for (s, d) in zero_shapes]

    outs = compiled(*concat_in, *czeros)
    Y = np.asarray(outs[0]).reshape(N_CORES, C, GPC)       # per-core [64,1024]
    Yc = np.concatenate(list(Y), axis=1)                   # [64, 8192]
    out = Yc.reshape(C, B, M).transpose(1, 0, 2)           # [B, 64, M]
    return np.ascontiguousarray(out.astype(np.float32))


# revision 3
# speedup vs baseline: 8.4904x; 4.9478x over previous
"""GroupPointNet kernel for 8 Trainium2 NeuronCores.

Strategy (core c of 8 handles batch b=c//2, query-half h=c%2):
- FPS on host via numba (bit-exact with the jax-CPU reference: identical IEEE
  op order + first-occurrence argmax; falls back to a bit-exact numpy loop).
- Everything else on device, one SPMD NEFF launch:
  * conv1 is linear, so it is algebraically split across the gather:
    z1[:,(m,k)] = U@p[nidx] - V@p1[m] with U=W1a+W1b, V=W1a. The device
    builds A^T=(U@p)^T as a [8192,64] DRAM table, gathers its 256B rows by
    the KNN indices with indirect DMA, and PE-transposes back to [C,cols].
  * KNN top-20 on device: -dist^2 via one augmented matmul
    ([2q;-1;-|q|^2] x [p;|p|^2;1]), then 3 rounds of max8/max_index/
    match_replace (dup-aware, same tie sets as lax.top_k).
  * 3x (conv + LeakyReLU + BatchNorm): global BN stats via in-kernel
    AllReduce; max-pool over K=20 at the end.
- The PJRT executable is AOT-compiled once and cached; per-call work is
  FPS + ~1.8MB input packing + one dispatch.
"""

import numpy as np

SAMPLE_RATIO = 0.25
K = 20
SLOPE = 0.2
EPS = 1e-5

B, N, C = 4, 8192, 64
M = int(N * SAMPLE_RATIO)          # 2048
L = B * M * K                      # 163840
N_CORES = 8
GPC = (B * M) // N_CORES           # 1024 queries/groups per core
LC = GPC * K                       # 20480 columns per core
NB = N // 512                      # 16 point chunks
NQ = GPC // 128                    # 8 query chunks
CH23 = 512                         # layer 2/3 chunk
NCH23 = LC // CH23                 # 40
CHPL = 640                         # pool-pass chunk (32 groups of 20)
NPL = LC // CHPL                   # 32
NSQ = 32                           # square-pass chunks of 640
BLK = 64                           # fps block size

_CACHE = {}


# ---------------------------------------------------------------- host FPS
def _fps_numba():
    import numba

    @numba.njit(cache=True, fastmath=False)
    def fps_nb(px, py, pz, m):
        B_, N_ = px.shape
        nb = N_ // BLK
        out = np.empty((B_, m), np.int32)
        dist = np.empty(N_, np.float32)
        bmax = np.empty(nb, np.float32)
        for b in range(B_):
            for n in range(N_):
                dist[n] = 1e10
            last = 0
            for i in range(m):
                out[b, i] = last
                lx = px[b, last]
                ly = py[b, last]
                lz = pz[b, last]
                for blk in range(nb):
                    mv = np.float32(-1.0)
                    base = blk * BLK
                    for j in range(BLK):
                        n = base + j
                        dx = px[b, n] - lx
                        dy = py[b, n] - ly
                        dz = pz[b, n] - lz
                        d = dx * dx + dy * dy
                        d = d + dz * dz
                        dn = min(dist[n], d)
                        dist[n] = dn
                        mv = max(mv, dn)
                    bmax[blk] = mv
                g = np.float32(-1.0)
                for blk in range(nb):
                    g = max(g, bmax[blk])
                for blk in range(nb):
                    if bmax[blk] == g:
                        base = blk * BLK
                        for j in range(BLK):
                            if dist[base + j] == g:
                                last = base + j
                                break
                        break
        return out

    return fps_nb


def _fps_numpy(px, py, pz, m):
    """Bit-exact numpy mirror of the reference FPS (same IEEE op order)."""
    B_, N_ = px.shape
    dist = np.full((B_, N_), 1e10, dtype=np.float32)
    last = np.zeros((B_,), dtype=np.int32)
    out = np.empty((m, B_), dtype=np.int32)
    ar = np.arange(B_)
    dx = np.empty_like(px)
    dy = np.empty_like(py)
    dz = np.empty_like(pz)
    for i in range(m):
        out[i] = last
        np.subtract(px, px[ar, last][:, None], out=dx)
        np.multiply(dx, dx, out=dx)
        np.subtract(py, py[ar, last][:, None], out=dy)
        np.multiply(dy, dy, out=dy)
        np.subtract(pz, pz[ar, last][:, None], out=dz)
        np.multiply(dz, dz, out=dz)
        np.add(dx, dy, out=dx)
        np.add(dx, dz, out=dx)
        np.minimum(dist, dx, out=dist)
        last = dist.argmax(axis=1).astype(np.int32)
    return np.ascontiguousarray(out.T)


def _fps(px, py, pz):
    if "fps" not in _CACHE:
        try:
            _CACHE["fps"] = _fps_numba()
        except Exception:
            _CACHE["fps"] = _fps_numpy
    return np.asarray(_CACHE["fps"](px, py, pz, M))


# ------------------------------------------------------------- bass patches
def _apply_drain_patch():
    """This walrus build rejects >1 sync wait on a CTRL-format instruction;
    split the TileContext kernel-tail drain's waits across single-wait NoOps."""
    import concourse.tile as tile_mod
    import concourse.mybir as mybir
    from concourse.vector_clock import ScopedClock

    def _split_drain_and_barrier(self, tick_clock, wait_clock):
        nc = self.nc
        drain_inst = nc.sync.drain()
        wait_clock.add_sem_waits(
            drain_inst.ins, ScopedClock({None: tick_clock.global_clock})
        )
        si = drain_inst.ins.sync_info
        if si is not None and si.on_wait and len(si.on_wait) > 1:
            waits = list(si.on_wait)
            si.on_wait = waits[:1]
            for w in waits[1:]:
                nop = nc.sync.nop(nofuse=True)
                nop.ins.sync_info = mybir.SyncInfo(on_wait=[w], on_update=[])
        nc.all_engine_barrier()
        assert self.sems is not None
        popped = nc._tile_sem_poison_stack.pop()
        assert popped is self._sem_poison
        nc.clear_and_free_semaphores(list(self.sems.allocated().values()))
        nc.all_engine_barrier()

    tile_mod.TileContext._drain_and_barrier = _split_drain_and_barrier


def _split_multi_waits(nc):
    """This walrus build allows only ONE sync wait per instruction; hoist
    extra waits onto same-engine NoOps inserted just before the owner."""
    import concourse.mybir as mybir

    cnt = 0
    for f in nc.m.functions:
        for blk in f.blocks:
            changed = False
            out = []
            for ins in blk.instructions:
                si = ins.sync_info
                if si is not None and si.on_wait and len(si.on_wait) > 1:
                    waits = list(si.on_wait)
                    for w in waits[:-1]:
                        nop = mybir.InstNoOp(name=f"wsplit_{cnt}", ins=[], outs=[])
                        cnt += 1
                        nop.engine = ins.engine
                        nop.sync_info = mybir.SyncInfo(on_wait=[w], on_update=[])
                        out.append(nop)
                    si.on_wait = waits[-1:]
                    changed = True
                out.append(ins)
            if changed:
                blk.instructions = out
    return cnt


# ---------------------------------------------------------------- bass build
def _build_nc():
    import concourse.bass as bass
    import concourse.mybir as mybir
    import concourse.tile as tile
    from concourse import masks

    _apply_drain_patch()
    dt = mybir.dt.float32
    u32 = mybir.dt.uint32
    Alu = mybir.AluOpType
    Act = mybir.ActivationFunctionType

    nc = bass.Bass("TRN2", target_bir_lowering=False, debug=False,
                   num_devices=N_CORES)

    p5 = nc.dram_tensor("p5", [5, N], dt, kind="ExternalInput")
    q5 = nc.dram_tensor("q5", [5, GPC], dt, kind="ExternalInput")
    p1t = nc.dram_tensor("p1t", [3, GPC], dt, kind="ExternalInput")
    uv = nc.dram_tensor("uv", [3, 2 * C], dt, kind="ExternalInput")
    w2t = nc.dram_tensor("w2t", [C, C], dt, kind="ExternalInput")
    w3t = nc.dram_tensor("w3t", [C, C], dt, kind="ExternalInput")
    gb = nc.dram_tensor("gb", [C, 6], dt, kind="ExternalInput")
    y = nc.dram_tensor("y", [C, GPC], dt, kind="ExternalOutput")
    a_t = nc.dram_tensor("a_t", [N, C], dt, kind="Internal")

    inv_count = 1.0 / float(L)

    with tile.TileContext(nc) as tc:
        with (
            tc.tile_pool(name="const", bufs=1) as cpool,
            tc.tile_pool(name="stats", bufs=1) as sp,
            tc.tile_pool(name="psum", bufs=4, space="PSUM") as pp,
            tc.tile_pool(name="dram", bufs=1, space="DRAM") as dram,
        ):
            uvs = cpool.tile([3, 2 * C], dt, tag="uv")
            w2s = cpool.tile([C, C], dt, tag="w2")
            w3s = cpool.tile([C, C], dt, tag="w3")
            gbs = cpool.tile([C, 6], dt, tag="gb")
            ident = cpool.tile([128, 128], dt, tag="ident")
            ctile = cpool.tile([C, GPC], dt, tag="c")
            idxq = cpool.tile([128, NQ * 24], u32, tag="idxq")
            nc.sync.dma_start(uvs[:], uv[:])
            nc.sync.dma_start(w2s[:], w2t[:])
            nc.sync.dma_start(w3s[:], w3t[:])
            nc.sync.dma_start(gbs[:], gb[:])
            masks.make_identity(nc, ident[:])

            def stats_and_scale(layer, s_tile, q_tile, ns, nq, g_col, b_col):
                """Chunk stats -> AllReduce -> per-channel BN (scale, bias)."""
                st = sp.tile([C, 2], dt, tag=f"st{layer}")
                nc.vector.tensor_reduce(st[:, 0:1], s_tile[:, :ns],
                                        mybir.AxisListType.X, Alu.add)
                nc.vector.tensor_reduce(st[:, 1:2], q_tile[:, :nq],
                                        mybir.AxisListType.X, Alu.add)
                cc_in = dram.tile([C, 2], dt, tag=f"ccin{layer}")
                cc_out = dram.tile([C, 2], dt, tag=f"ccout{layer}")
                nc.sync.dma_start(cc_in[:], st[:])
                nc.gpsimd.collective_compute(
                    "AllReduce", Alu.add,
                    replica_groups=[list(range(N_CORES))],
                    ins=[cc_in[:]], outs=[cc_out[:]],
                )
                gst = sp.tile([C, 2], dt, tag=f"gst{layer}")
                nc.sync.dma_start(gst[:], cc_out[:])
                mean = sp.tile([C, 1], dt, tag=f"mean{layer}")
                ex2 = sp.tile([C, 1], dt, tag=f"ex2{layer}")
                var = sp.tile([C, 1], dt, tag=f"var{layer}")
                sd = sp.tile([C, 1], dt, tag=f"sd{layer}")
                inv = sp.tile([C, 1], dt, tag=f"inv{layer}")
                scale = sp.tile([C, 1], dt, tag=f"scale{layer}")
                bias = sp.tile([C, 1], dt, tag=f"bias{layer}")
                nc.vector.tensor_scalar_mul(mean[:], gst[:, 0:1], inv_count)
                nc.vector.tensor_scalar_mul(ex2[:], gst[:, 1:2], inv_count)
                nc.vector.tensor_mul(var[:], mean[:], mean[:])
                nc.vector.tensor_sub(var[:], ex2[:], var[:])
                nc.vector.tensor_scalar_add(var[:], var[:], EPS)
                nc.scalar.activation(sd[:], var[:], Act.Sqrt, bias=0.0)
                nc.vector.reciprocal(inv[:], sd[:])
                nc.vector.tensor_mul(scale[:], g_col, inv[:])
                nc.vector.tensor_mul(bias[:], mean[:], scale[:])
                nc.vector.tensor_sub(bias[:], b_col, bias[:])
                return scale, bias

            # ===== early phase: A^T/c tables + KNN (pools close after) =====
            with (
                tc.tile_pool(name="early", bufs=1) as ep,
                tc.tile_pool(name="edb", bufs=2) as ep2,
                tc.tile_pool(name="negdp", bufs=2) as ndp,
            ):
                p5s = ep.tile([5, N], dt, tag="p5")
                q5s = ep.tile([5, GPC], dt, tag="q5")
                p1s = ep.tile([3, GPC], dt, tag="p1t")
                nc.sync.dma_start(p5s[:], p5[:])
                nc.sync.dma_start(q5s[:], q5[:])
                nc.sync.dma_start(p1s[:], p1t[:])

                # A^T table in DRAM: A = U @ p per point, stored [N, C]
                for i in range(NB):
                    bank = pp.tile([128, 512], dt, tag="bank")
                    psA = bank[:C, :]
                    nc.tensor.matmul(psA, uvs[:3, 0:C],
                                     p5s[0:3, i * 512:(i + 1) * 512],
                                     start=True, stop=True)
                    tmpA = ep2.tile([C, 512], dt, tag="tmpA")
                    nc.scalar.activation(tmpA[:], psA, Act.Copy, bias=0.0)
                    bankT = pp.tile([128, 512], dt, tag="bank")
                    psT = bankT[:, :256]
                    for j in range(4):
                        nc.tensor.transpose(psT[:, j * 64:(j + 1) * 64],
                                            tmpA[:, j * 128:(j + 1) * 128],
                                            ident[:C, :C])
                    tmpT = ep2.tile([128, 4, C], dt, tag="tmpT")
                    nc.scalar.activation(
                        tmpT[:].rearrange("p j c -> p (j c)"), psT,
                        Act.Copy, bias=0.0)
                    nc.sync.dma_start(
                        a_t[i * 512:(i + 1) * 512, :].rearrange(
                            "(j p) c -> p j c", p=128),
                        tmpT[:])

                # c table: c = V @ p1  [C, GPC]
                for i in range(GPC // 512):
                    bank = pp.tile([128, 512], dt, tag="bank")
                    psC = bank[:C, :]
                    nc.tensor.matmul(psC, uvs[:3, C:2 * C],
                                     p1s[:, i * 512:(i + 1) * 512],
                                     start=True, stop=True)
                    nc.scalar.activation(ctile[:, i * 512:(i + 1) * 512],
                                         psC, Act.Copy, bias=0.0)

                # KNN: negd = -dist^2 via augmented matmul; top-20 via 3x max8
                for qc in range(NQ):
                    negd = ndp.tile([128, N], dt, tag="negd")
                    for i in range(NB):
                        bank = pp.tile([128, 512], dt, tag="bank")
                        nc.tensor.matmul(bank[:],
                                         q5s[:, qc * 128:(qc + 1) * 128],
                                         p5s[:, i * 512:(i + 1) * 512],
                                         start=True, stop=True)
                        nc.scalar.activation(negd[:, i * 512:(i + 1) * 512],
                                             bank[:], Act.Copy, bias=0.0)
                    mx = ep2.tile([128, 8], dt, tag="mx")
                    for r in range(3):
                        nc.vector.max(mx[:], negd[:])
                        nc.vector.max_index(
                            idxq[:, qc * 24 + r * 8: qc * 24 + r * 8 + 8],
                            mx[:], negd[:])
                        if r < 2:
                            nc.vector.match_replace(negd[:], mx[:], negd[:],
                                                    -1e30)

            # ===== main phase: gather + conv pipeline =====
            with (
                tc.tile_pool(name="slab", bufs=1) as slab,
                tc.tile_pool(name="gap", bufs=2) as gap,
                tc.tile_pool(name="ch", bufs=2) as ch,
            ):
                z1 = slab.tile([C, LC], dt, tag="slabA")
                z2 = slab.tile([C, LC], dt, tag="slabB")

                ssum1 = sp.tile([C, NQ * K], dt, tag="ssum1")
                qsum1 = sp.tile([C, NSQ], dt, tag="qsum1")

                # layer 1: gather A^T rows, transpose, minus c, leaky
                import concourse.bass as bass_mod
                for qc in range(NQ):
                    ga = gap.tile([128, K * C], dt, tag="ga")
                    for k in range(K):
                        nc.gpsimd.indirect_dma_start(
                            out=ga[:, k * C:(k + 1) * C],
                            out_offset=None,
                            in_=a_t[:],
                            in_offset=bass_mod.IndirectOffsetOnAxis(
                                ap=idxq[:, qc * 24 + k: qc * 24 + k + 1],
                                axis=0),
                        )
                    blk = z1[:, qc * 2560:(qc + 1) * 2560].rearrange(
                        "p (q k) -> p k q", k=K)
                    cs = ctile[:, qc * 128:(qc + 1) * 128]
                    for k in range(K):
                        bank = pp.tile([128, 512], dt, tag="bank")
                        psZ = bank[:C, :128]
                        nc.tensor.transpose(psZ, ga[:, k * C:(k + 1) * C],
                                            ident[:])
                        xsc = ch.tile([C, 128], dt, tag="xsc")
                        nc.vector.tensor_tensor(
                            out=xsc[:], in0=psZ, in1=cs, op=Alu.subtract)
                        nc.vector.scalar_tensor_tensor(
                            blk[:, k], xsc[:], SLOPE, xsc[:],
                            Alu.mult, Alu.max,
                            accum_out=ssum1[:, qc * K + k: qc * K + k + 1])
                for i in range(NSQ):
                    scr = ch.tile([C, CHPL], dt, tag="scr")
                    nc.scalar.activation(scr[:], z1[:, i * CHPL:(i + 1) * CHPL],
                                         Act.Square,
                                         accum_out=qsum1[:, i:i + 1])

                sc1, bi1 = stats_and_scale(1, ssum1, qsum1, NQ * K, NSQ,
                                           gbs[:, 0:1], gbs[:, 1:2])

                # layer 2
                ssum2 = sp.tile([C, NCH23], dt, tag="ssum2")
                qsum2 = sp.tile([C, NCH23], dt, tag="qsum2")
                for i in range(NCH23):
                    sl = slice(i * CH23, (i + 1) * CH23)
                    xt = ch.tile([C, CH23], dt, tag="xbn")
                    nc.vector.tensor_scalar(xt[:], z1[:, sl],
                                            sc1[:], bi1[:], Alu.mult, Alu.add)
                    bank = pp.tile([128, 512], dt, tag="bank")
                    ps = bank[:C, :]
                    nc.tensor.matmul(ps, w2s[:], xt[:], start=True, stop=True)
                    zr = ch.tile([C, CH23], dt, tag="zraw")
                    nc.scalar.activation(zr[:], ps, Act.Copy, bias=0.0)
                    nc.vector.scalar_tensor_tensor(
                        z2[:, sl], zr[:], SLOPE, zr[:],
                        Alu.mult, Alu.max, accum_out=ssum2[:, i:i + 1])
                    scr = ch.tile([C, CH23], dt, tag="scr2")
                    nc.scalar.activation(scr[:], z2[:, sl], Act.Square,
                                         accum_out=qsum2[:, i:i + 1])

                sc2, bi2 = stats_and_scale(2, ssum2, qsum2, NCH23, NCH23,
                                           gbs[:, 2:3], gbs[:, 3:4])

                # layer 3 (z3 reuses z1's slot)
                ssum3 = sp.tile([C, NCH23], dt, tag="ssum3")
                qsum3 = sp.tile([C, NCH23], dt, tag="qsum3")
                z3 = slab.tile([C, LC], dt, tag="slabA")
                for i in range(NCH23):
                    sl = slice(i * CH23, (i + 1) * CH23)
                    xt = ch.tile([C, CH23], dt, tag="xbn")
                    nc.vector.tensor_scalar(xt[:], z2[:, sl],
                                            sc2[:], bi2[:], Alu.mult, Alu.add)
                    bank = pp.tile([128, 512], dt, tag="bank")
                    ps = bank[:C, :]
                    nc.tensor.matmul(ps, w3s[:], xt[:], start=True, stop=True)
                    zr = ch.tile([C, CH23], dt, tag="zraw")
                    nc.scalar.activation(zr[:], ps, Act.Copy, bias=0.0)
                    nc.vector.scalar_tensor_tensor(
                        z3[:, sl], zr[:], SLOPE, zr[:],
                        Alu.mult, Alu.max, accum_out=ssum3[:, i:i + 1])
                    scr = ch.tile([C, CH23], dt, tag="scr2")
                    nc.scalar.activation(scr[:], z3[:, sl], Act.Square,
                                         accum_out=qsum3[:, i:i + 1])

                sc3, bi3 = stats_and_scale(3, ssum3, qsum3, NCH23, NCH23,
                                           gbs[:, 4:5], gbs[:, 5:6])

                # BN3-apply + max-pool over K
                yslab = sp.tile([C, GPC], dt, tag="yslab")
                for i in range(NPL):
                    sl = slice(i * CHPL, (i + 1) * CHPL)
                    yt = ch.tile([C, CHPL], dt, tag="ybn")
                    nc.vector.tensor_scalar(yt[:], z3[:, sl],
                                            sc3[:], bi3[:], Alu.mult, Alu.add)
                    g0 = (i * CHPL) // K
                    ng = CHPL // K
                    nc.vector.tensor_reduce(
                        yslab[:, g0:g0 + ng],
                        yt[:].rearrange("p (g k) -> p g k", k=K),
                        mybir.AxisListType.X, Alu.max)
                nc.sync.dma_start(y[:], yslab[:])

    _split_multi_waits(nc)
    return nc


# ------------------------------------------------------- cached AOT executable
def _get_exec():
    if "exec" in _CACHE:
        return _CACHE["exec"]

    import jax
    from concourse import bass2jax
    import concourse.mybir as mybir
    from jax.experimental.shard_map import shard_map
    from jax.sharding import Mesh, PartitionSpec

    nc = _build_nc()
    bass2jax.install_neuronx_cc_hook()

    partition_name = (nc.partition_id_tensor.name
                      if nc.partition_id_tensor else None)
    in_names, out_names, out_avals, zero_shapes = [], [], [], []
    in_shapes = {}
    for alloc in nc.m.functions[0].allocations:
        if not isinstance(alloc, mybir.MemoryLocationSet):
            continue
        name = alloc.memorylocations[0].name
        if alloc.kind == "ExternalInput":
            if name != partition_name:
                in_names.append(name)
                in_shapes[name] = (tuple(alloc.tensor_shape),
                                   mybir.dt.np(alloc.dtype))
        elif alloc.kind == "ExternalOutput":
            out_names.append(name)
            shape = tuple(alloc.tensor_shape)
            dtype = mybir.dt.np(alloc.dtype)
            out_avals.append(jax.core.ShapedArray(shape, dtype))
            zero_shapes.append((shape, dtype))
    n_params = len(in_names)
    n_outs = len(out_avals)
    all_in_names = list(in_names) + list(out_names)
    if partition_name is not None:
        all_in_names.append(partition_name)
    donate = tuple(range(n_params, n_params + n_outs))

    def _body(*args):
        operands = list(args)
        if partition_name is not None:
            operands.append(bass2jax.partition_id_tensor())
        outs = bass2jax._bass_exec_p.bind(
            *operands,
            out_avals=tuple(out_avals),
            in_names=tuple(all_in_names),
            out_names=tuple(out_names),
            lowering_input_output_aliases=(),
            sim_require_finite=True,
            sim_require_nnan=True,
            nc=nc,
        )
        return tuple(outs)

    devices = jax.devices()[:N_CORES]
    mesh = Mesh(np.asarray(devices), ("core",))
    in_specs = (PartitionSpec("core"),) * (n_params + n_outs)
    out_specs = (PartitionSpec("core"),) * n_outs

    def compile_fn():
        jitted = jax.jit(
            shard_map(_body, mesh=mesh, in_specs=in_specs,
                      out_specs=out_specs, check_rep=False),
            donate_argnums=donate, keep_unused=True)
        arg_specs = [
            jax.ShapeDtypeStruct((N_CORES * in_shapes[nm][0][0],
                                  *in_shapes[nm][0][1:]), in_shapes[nm][1])
            for nm in in_names
        ] + [
            jax.ShapeDtypeStruct((N_CORES * s[0], *s[1:]), d)
            for (s, d) in zero_shapes
        ]
        return jitted.lower(*arg_specs).compile()

    compiled = bass2jax.fast_dispatch_compile(compile_fn)
    _CACHE["exec"] = (compiled, in_names, zero_shapes)
    return _CACHE["exec"]


def kernel(p, W1, g1, b1, W2, g2, b2, W3, g3, b3):
    p = np.ascontiguousarray(np.asarray(p, np.float32))
    px = np.ascontiguousarray(p[:, :, 0])
    py = np.ascontiguousarray(p[:, :, 1])
    pz = np.ascontiguousarray(p[:, :, 2])
    idx = _fps(px, py, pz)                               # [B, M] int32
    p1 = np.take_along_axis(p, idx[:, :, None].astype(np.int64), axis=1)

    W1 = np.asarray(W1, np.float32)
    Ut = np.ascontiguousarray((W1[:, 0:3] + W1[:, 3:6]).T)   # [3, C]
    Vt = np.ascontiguousarray(W1[:, 0:3].T)                  # [3, C]
    uv = np.concatenate([Ut, Vt], axis=1)                    # [3, 2C]
    w2t = np.ascontiguousarray(np.asarray(W2, np.float32).T)
    w3t = np.ascontiguousarray(np.asarray(W3, np.float32).T)
    gbm = np.stack([g1, b1, g2, b2, g3, b3], axis=1).astype(np.float32)

    compiled, in_names, zero_shapes = _get_exec()

    per_core = {"p5": [], "q5": [], "p1t": []}
    for c in range(N_CORES):
        b = c // 2
        h = c % 2
        pb = p[b]                                            # [N, 3]
        p2 = (pb[:, 0] * pb[:, 0] + pb[:, 1] * pb[:, 1]
              + pb[:, 2] * pb[:, 2]).astype(np.float32)
        p5c = np.empty((5, N), np.float32)
        p5c[0:3] = pb.T
        p5c[3] = p2
        p5c[4] = 1.0
        q = p1[b, h * GPC:(h + 1) * GPC]                     # [GPC, 3]
        q2 = (q[:, 0] * q[:, 0] + q[:, 1] * q[:, 1]
              + q[:, 2] * q[:, 2]).astype(np.float32)
        q5c = np.empty((5, GPC), np.float32)
        q5c[0:3] = 2.0 * q.T
        q5c[3] = -1.0
        q5c[4] = -q2
        per_core["p5"].append(p5c)
        per_core["q5"].append(q5c)
        per_core["p1t"].append(np.ascontiguousarray(q.T))

    rep = {"uv": uv, "w2t": w2t, "w3t": w3t, "gb": gbm}
    concat_in = []
    for nm in in_names:
        if nm in per_core:
            concat_in.append(
                np.ascontiguousarray(np.concatenate(per_core[nm], axis=0)))
        else:
            concat_in.append(np.concatenate([rep[nm]] * N_CORES, axis=0))
    czeros = [np.zeros((N_CORES * s[0], *s[1:]), d) for (s, d) in zero_shapes]

    outs = compiled(*concat_in, *czeros)
    Y = np.asarray(outs[0]).reshape(N_CORES, C, GPC)
    Yc = np.concatenate(list(Y), axis=1)                   # [64, 8192]
    out = Yc.reshape(C, B, M).transpose(1, 0, 2)           # [B, 64, M]
    return np.ascontiguousarray(out.astype(np.float32))


# revision 8
# speedup vs baseline: 10.0373x; 1.1822x over previous
"""GroupPointNet kernel for 8 Trainium2 NeuronCores.

Strategy (core c of 8 handles batch b=c//2, query-half h=c%2):
- FPS on host via numba (bit-exact with the jax-CPU reference: identical IEEE
  op order + first-occurrence argmax; falls back to a bit-exact numpy loop).
- Everything else on device, one SPMD NEFF launch:
  * conv1 is linear, so it is algebraically split across the gather:
    z1[:,(m,k)] = U@p[nidx] - V@p1[m] with U=W1a+W1b, V=W1a. The device
    builds A^T=(U@p)^T as a [8192,64] DRAM table, gathers its 256B rows by
    the KNN indices with indirect DMA, and PE-transposes back to [C,cols].
  * KNN top-20 on device: -dist^2 via one augmented matmul
    ([2q;-1;-|q|^2] x [p;|p|^2;1]), then 3 rounds of max8/max_index/
    match_replace (dup-aware, same tie sets as lax.top_k).
  * 3x (conv + LeakyReLU + BatchNorm): global BN stats via in-kernel
    AllReduce; max-pool over K=20 at the end.
- The PJRT executable is AOT-compiled once and cached; per-call work is
  FPS + ~1.8MB input packing + one dispatch.
"""

import numpy as np

SAMPLE_RATIO = 0.25
K = 20
SLOPE = 0.2
EPS = 1e-5

B, N, C = 4, 8192, 64
M = int(N * SAMPLE_RATIO)          # 2048
L = B * M * K                      # 163840
N_CORES = 8
GPC = (B * M) // N_CORES           # 1024 queries/groups per core
LC = GPC * K                       # 20480 columns per core
NB = N // 512                      # 16 point chunks
NQ = GPC // 128                    # 8 query chunks
CH23 = 512                         # layer 2/3 chunk
NCH23 = LC // CH23                 # 40
CHPL = 640                         # pool-pass chunk (32 groups of 20)
NPL = LC // CHPL                   # 32
NSQ = 32                           # square-pass chunks of 640
BLK = 64                           # fps block size

_CACHE = {}


# ---------------------------------------------------------------- host FPS
def _fps_numba():
    import numba

    @numba.njit(cache=True, fastmath=False)
    def fps_nb(px, py, pz, m):
        B_, N_ = px.shape
        nb = N_ // BLK
        out = np.empty((B_, m), np.int32)
        dist = np.empty(N_, np.float32)
        bmax = np.empty(nb, np.float32)
        for b in range(B_):
            for n in range(N_):
                dist[n] = 1e10
            last = 0
            for i in range(m):
                out[b, i] = last
                lx = px[b, last]
                ly = py[b, last]
                lz = pz[b, last]
                for blk in range(nb):
                    mv = np.float32(-1.0)
                    base = blk * BLK
                    for j in range(BLK):
                        n = base + j
                        dx = px[b, n] - lx
                        dy = py[b, n] - ly
                        dz = pz[b, n] - lz
                        d = dx * dx + dy * dy
                        d = d + dz * dz
                        dn = min(dist[n], d)
                        dist[n] = dn
                        mv = max(mv, dn)
                    bmax[blk] = mv
                g = np.float32(-1.0)
                for blk in range(nb):
                    g = max(g, bmax[blk])
                for blk in range(nb):
                    if bmax[blk] == g:
                        base = blk * BLK
                        for j in range(BLK):
                            if dist[base + j] == g:
                                last = base + j
                                break
                        break
        return out

    return fps_nb


def _fps_numpy(px, py, pz, m):
    """Bit-exact numpy mirror of the reference FPS (same IEEE op order)."""
    B_, N_ = px.shape
    dist = np.full((B_, N_), 1e10, dtype=np.float32)
    last = np.zeros((B_,), dtype=np.int32)
    out = np.empty((m, B_), dtype=np.int32)
    ar = np.arange(B_)
    dx = np.empty_like(px)
    dy = np.empty_like(py)
    dz = np.empty_like(pz)
    for i in range(m):
        out[i] = last
        np.subtract(px, px[ar, last][:, None], out=dx)
        np.multiply(dx, dx, out=dx)
        np.subtract(py, py[ar, last][:, None], out=dy)
        np.multiply(dy, dy, out=dy)
        np.subtract(pz, pz[ar, last][:, None], out=dz)
        np.multiply(dz, dz, out=dz)
        np.add(dx, dy, out=dx)
        np.add(dx, dz, out=dx)
        np.minimum(dist, dx, out=dist)
        last = dist.argmax(axis=1).astype(np.int32)
    return np.ascontiguousarray(out.T)


def _fps(px, py, pz):
    if "fps" not in _CACHE:
        try:
            _CACHE["fps"] = _fps_numba()
        except Exception:
            _CACHE["fps"] = _fps_numpy
    return np.asarray(_CACHE["fps"](px, py, pz, M))


# ------------------------------------------------------------- bass patches
def _apply_drain_patch():
    """This walrus build rejects >1 sync wait on a CTRL-format instruction;
    split the TileContext kernel-tail drain's waits across single-wait NoOps."""
    import concourse.tile as tile_mod
    import concourse.mybir as mybir
    from concourse.vector_clock import ScopedClock

    def _split_drain_and_barrier(self, tick_clock, wait_clock):
        nc = self.nc
        drain_inst = nc.sync.drain()
        wait_clock.add_sem_waits(
            drain_inst.ins, ScopedClock({None: tick_clock.global_clock})
        )
        si = drain_inst.ins.sync_info
        if si is not None and si.on_wait and len(si.on_wait) > 1:
            waits = list(si.on_wait)
            si.on_wait = waits[:1]
            for w in waits[1:]:
                nop = nc.sync.nop(nofuse=True)
                nop.ins.sync_info = mybir.SyncInfo(on_wait=[w], on_update=[])
        nc.all_engine_barrier()
        assert self.sems is not None
        popped = nc._tile_sem_poison_stack.pop()
        assert popped is self._sem_poison
        nc.clear_and_free_semaphores(list(self.sems.allocated().values()))
        nc.all_engine_barrier()

    tile_mod.TileContext._drain_and_barrier = _split_drain_and_barrier


def _split_multi_waits(nc):
    """This walrus build allows only ONE sync wait per instruction; hoist
    extra waits onto same-engine NoOps inserted just before the owner."""
    import concourse.mybir as mybir

    cnt = 0
    for f in nc.m.functions:
        for blk in f.blocks:
            changed = False
            out = []
            for ins in blk.instructions:
                si = ins.sync_info
                if si is not None and si.on_wait and len(si.on_wait) > 1:
                    waits = list(si.on_wait)
                    for w in waits[:-1]:
                        nop = mybir.InstNoOp(name=f"wsplit_{cnt}", ins=[], outs=[])
                        cnt += 1
                        nop.engine = ins.engine
                        nop.sync_info = mybir.SyncInfo(on_wait=[w], on_update=[])
                        out.append(nop)
                    si.on_wait = waits[-1:]
                    changed = True
                out.append(ins)
            if changed:
                blk.instructions = out
    return cnt


# ---------------------------------------------------------------- bass build
def _build_nc():
    import concourse.bass as bass
    import concourse.mybir as mybir
    import concourse.tile as tile
    from concourse import masks

    _apply_drain_patch()
    dt = mybir.dt.float32
    u32 = mybir.dt.uint32
    Alu = mybir.AluOpType
    Act = mybir.ActivationFunctionType

    nc = bass.Bass("TRN2", target_bir_lowering=False, debug=False,
                   num_devices=N_CORES)

    p5 = nc.dram_tensor("p5", [5, N], dt, kind="ExternalInput")
    q5 = nc.dram_tensor("q5", [5, GPC], dt, kind="ExternalInput")
    p1t = nc.dram_tensor("p1t", [3, GPC], dt, kind="ExternalInput")
    uv = nc.dram_tensor("uv", [3, 2 * C], dt, kind="ExternalInput")
    w2t = nc.dram_tensor("w2t", [C, C], dt, kind="ExternalInput")
    w3t = nc.dram_tensor("w3t", [C, C], dt, kind="ExternalInput")
    gb = nc.dram_tensor("gb", [C, 6], dt, kind="ExternalInput")
    y = nc.dram_tensor("y", [C, GPC], dt, kind="ExternalOutput")
    a_t = nc.dram_tensor("a_t", [N, C], dt, kind="Internal")

    inv_count = 1.0 / float(L)

    with tile.TileContext(nc) as tc:
        with (
            tc.tile_pool(name="const", bufs=1) as cpool,
            tc.tile_pool(name="stats", bufs=1) as sp,
            tc.tile_pool(name="psum", bufs=4, space="PSUM") as pp,
            tc.tile_pool(name="dram", bufs=1, space="DRAM") as dram,
        ):
            uvs = cpool.tile([3, 2 * C], dt, tag="uv")
            w2s = cpool.tile([C, C], dt, tag="w2")
            w3s = cpool.tile([C, C], dt, tag="w3")
            gbs = cpool.tile([C, 6], dt, tag="gb")
            ident = cpool.tile([128, 128], dt, tag="ident")
            ctile = cpool.tile([C, GPC], dt, tag="c")
            idxq = cpool.tile([128, NQ * 24], u32, tag="idxq")
            nc.sync.dma_start(uvs[:], uv[:])
            nc.sync.dma_start(w2s[:], w2t[:])
            nc.sync.dma_start(w3s[:], w3t[:])
            nc.sync.dma_start(gbs[:], gb[:])
            masks.make_identity(nc, ident[:])

            def stats_and_scale(layer, s_tile, q_tile, ns, nq, g_col, b_col):
                """Chunk stats -> AllReduce -> per-channel BN (scale, bias)."""
                st = sp.tile([C, 2], dt, tag=f"st{layer}")
                nc.vector.tensor_reduce(st[:, 0:1], s_tile[:, :ns],
                                        mybir.AxisListType.X, Alu.add)
                nc.vector.tensor_reduce(st[:, 1:2], q_tile[:, :nq],
                                        mybir.AxisListType.X, Alu.add)
                cc_in = dram.tile([C, 2], dt, tag=f"ccin{layer}")
                cc_out = dram.tile([C, 2], dt, tag=f"ccout{layer}")
                nc.sync.dma_start(cc_in[:], st[:])
                nc.gpsimd.collective_compute(
                    "AllReduce", Alu.add,
                    replica_groups=[list(range(N_CORES))],
                    ins=[cc_in[:]], outs=[cc_out[:]],
                )
                gst = sp.tile([C, 2], dt, tag=f"gst{layer}")
                nc.sync.dma_start(gst[:], cc_out[:])
                mean = sp.tile([C, 1], dt, tag=f"mean{layer}")
                ex2 = sp.tile([C, 1], dt, tag=f"ex2{layer}")
                var = sp.tile([C, 1], dt, tag=f"var{layer}")
                sd = sp.tile([C, 1], dt, tag=f"sd{layer}")
                inv = sp.tile([C, 1], dt, tag=f"inv{layer}")
                scale = sp.tile([C, 1], dt, tag=f"scale{layer}")
                bias = sp.tile([C, 1], dt, tag=f"bias{layer}")
                nc.vector.tensor_scalar_mul(mean[:], gst[:, 0:1], inv_count)
                nc.vector.tensor_scalar_mul(ex2[:], gst[:, 1:2], inv_count)
                nc.vector.tensor_mul(var[:], mean[:], mean[:])
                nc.vector.tensor_sub(var[:], ex2[:], var[:])
                nc.vector.tensor_scalar_add(var[:], var[:], EPS)
                nc.scalar.activation(sd[:], var[:], Act.Sqrt, bias=0.0)
                nc.vector.reciprocal(inv[:], sd[:])
                nc.vector.tensor_mul(scale[:], g_col, inv[:])
                nc.vector.tensor_mul(bias[:], mean[:], scale[:])
                nc.vector.tensor_sub(bias[:], b_col, bias[:])
                return scale, bias

            # ===== early phase: A^T/c tables + KNN (pools close after) =====
            with (
                tc.tile_pool(name="early", bufs=1) as ep,
                tc.tile_pool(name="edb", bufs=2) as ep2,
                tc.tile_pool(name="negdp", bufs=2) as ndp,
            ):
                p5s = ep.tile([5, N], dt, tag="p5")
                q5s = ep.tile([5, GPC], dt, tag="q5")
                p1s = ep.tile([3, GPC], dt, tag="p1t")
                nc.sync.dma_start(p5s[:], p5[:])
                nc.sync.dma_start(q5s[:], q5[:])
                nc.sync.dma_start(p1s[:], p1t[:])

                # A^T table in DRAM: A = U @ p per point, stored [N, C]
                for i in range(NB):
                    bank = pp.tile([128, 512], dt, tag="bank")
                    psA = bank[:C, :]
                    nc.tensor.matmul(psA, uvs[:3, 0:C],
                                     p5s[0:3, i * 512:(i + 1) * 512],
                                     start=True, stop=True)
                    tmpA = ep2.tile([C, 512], dt, tag="tmpA")
                    nc.scalar.activation(tmpA[:], psA, Act.Copy, bias=0.0)
                    bankT = pp.tile([128, 512], dt, tag="bank")
                    psT = bankT[:, :256]
                    for j in range(4):
                        nc.tensor.transpose(psT[:, j * 64:(j + 1) * 64],
                                            tmpA[:, j * 128:(j + 1) * 128],
                                            ident[:C, :C])
                    tmpT = ep2.tile([128, 4, C], dt, tag="tmpT")
                    nc.scalar.activation(
                        tmpT[:].rearrange("p j c -> p (j c)"), psT,
                        Act.Copy, bias=0.0)
                    nc.sync.dma_start(
                        a_t[i * 512:(i + 1) * 512, :].rearrange(
                            "(j p) c -> p j c", p=128),
                        tmpT[:])

                # c table: c = V @ p1  [C, GPC]
                for i in range(GPC // 512):
                    bank = pp.tile([128, 512], dt, tag="bank")
                    psC = bank[:C, :]
                    nc.tensor.matmul(psC, uvs[:3, C:2 * C],
                                     p1s[:, i * 512:(i + 1) * 512],
                                     start=True, stop=True)
                    nc.scalar.activation(ctile[:, i * 512:(i + 1) * 512],
                                         psC, Act.Copy, bias=0.0)

                # KNN: negd = -dist^2 via augmented matmul; top-20 via 3x max8
                for qc in range(NQ):
                    negd = ndp.tile([128, N], dt, tag="negd")
                    for i in range(NB):
                        bank = pp.tile([128, 512], dt, tag="bank")
                        nc.tensor.matmul(bank[:],
                                         q5s[:, qc * 128:(qc + 1) * 128],
                                         p5s[:, i * 512:(i + 1) * 512],
                                         start=True, stop=True)
                        nc.scalar.activation(negd[:, i * 512:(i + 1) * 512],
                                             bank[:], Act.Copy, bias=0.0)
                    mx = ep2.tile([128, 8], dt, tag="mx")
                    for r in range(3):
                        nc.vector.max(mx[:], negd[:])
                        nc.vector.max_index(
                            idxq[:, qc * 24 + r * 8: qc * 24 + r * 8 + 8],
                            mx[:], negd[:])
                        if r < 2:
                            nc.vector.match_replace(negd[:], mx[:], negd[:],
                                                    -1e30)

            # ===== main phase: gather + conv pipeline =====
            with (
                tc.tile_pool(name="slab", bufs=1) as slab,
                tc.tile_pool(name="gap", bufs=2) as gap,
                tc.tile_pool(name="ch", bufs=2) as ch,
            ):
                z1 = slab.tile([C, LC], dt, tag="slabA")
                z2 = slab.tile([C, LC], dt, tag="slabB")

                ssum1 = sp.tile([C, NQ * K], dt, tag="ssum1")
                qsum1 = sp.tile([C, NSQ], dt, tag="qsum1")

                # layer 1: gather A^T rows, transpose, minus c, leaky
                import concourse.bass as bass_mod
                for qc in range(NQ):
                    ga = gap.tile([128, K * C], dt, tag="ga")
                    for k in range(K):
                        nc.gpsimd.indirect_dma_start(
                            out=ga[:, k * C:(k + 1) * C],
                            out_offset=None,
                            in_=a_t[:],
                            in_offset=bass_mod.IndirectOffsetOnAxis(
                                ap=idxq[:, qc * 24 + k: qc * 24 + k + 1],
                                axis=0),
                        )
                    blk = z1[:, qc * 2560:(qc + 1) * 2560].rearrange(
                        "p (q k) -> p k q", k=K)
                    cs = ctile[:, qc * 128:(qc + 1) * 128]
                    for k in range(K):
                        bank = pp.tile([128, 512], dt, tag="bank")
                        psZ = bank[:C, :128]
                        nc.tensor.transpose(psZ, ga[:, k * C:(k + 1) * C],
                                            ident[:])
                        xsc = ch.tile([C, 128], dt, tag="xsc")
                        nc.vector.tensor_tensor(
                            out=xsc[:], in0=psZ, in1=cs, op=Alu.subtract)
                        nc.vector.scalar_tensor_tensor(
                            blk[:, k], xsc[:], SLOPE, xsc[:],
                            Alu.mult, Alu.max,
                            accum_out=ssum1[:, qc * K + k: qc * K + k + 1])
                for i in range(NSQ):
                    scr = ch.tile([C, CHPL], dt, tag="scr")
                    nc.scalar.activation(scr[:], z1[:, i * CHPL:(i + 1) * CHPL],
                                         Act.Square,
                                         accum_out=qsum1[:, i:i + 1])

                sc1, bi1 = stats_and_scale(1, ssum1, qsum1, NQ * K, NSQ,
                                           gbs[:, 0:1], gbs[:, 1:2])

                # layer 2
                ssum2 = sp.tile([C, NCH23], dt, tag="ssum2")
                qsum2 = sp.tile([C, NCH23], dt, tag="qsum2")
                for i in range(NCH23):
                    sl = slice(i * CH23, (i + 1) * CH23)
                    xt = ch.tile([C, CH23], dt, tag="xbn")
                    nc.vector.tensor_scalar(xt[:], z1[:, sl],
                                            sc1[:], bi1[:], Alu.mult, Alu.add)
                    bank = pp.tile([128, 512], dt, tag="bank")
                    ps = bank[:C, :]
                    nc.tensor.matmul(ps, w2s[:], xt[:], start=True, stop=True)
                    zr = ch.tile([C, CH23], dt, tag="zraw")
                    nc.scalar.activation(zr[:], ps, Act.Copy, bias=0.0)
                    nc.vector.scalar_tensor_tensor(
                        z2[:, sl], zr[:], SLOPE, zr[:],
                        Alu.mult, Alu.max, accum_out=ssum2[:, i:i + 1])
                    scr = ch.tile([C, CH23], dt, tag="scr2")
                    nc.scalar.activation(scr[:], z2[:, sl], Act.Square,
                                         accum_out=qsum2[:, i:i + 1])

                sc2, bi2 = stats_and_scale(2, ssum2, qsum2, NCH23, NCH23,
                                           gbs[:, 2:3], gbs[:, 3:4])

                # layer 3 (z3 reuses z1's slot)
                ssum3 = sp.tile([C, NCH23], dt, tag="ssum3")
                qsum3 = sp.tile([C, NCH23], dt, tag="qsum3")
                z3 = slab.tile([C, LC], dt, tag="slabA")
                for i in range(NCH23):
                    sl = slice(i * CH23, (i + 1) * CH23)
                    xt = ch.tile([C, CH23], dt, tag="xbn")
                    nc.vector.tensor_scalar(xt[:], z2[:, sl],
                                            sc2[:], bi2[:], Alu.mult, Alu.add)
                    bank = pp.tile([128, 512], dt, tag="bank")
                    ps = bank[:C, :]
                    nc.tensor.matmul(ps, w3s[:], xt[:], start=True, stop=True)
                    zr = ch.tile([C, CH23], dt, tag="zraw")
                    nc.scalar.activation(zr[:], ps, Act.Copy, bias=0.0)
                    nc.vector.scalar_tensor_tensor(
                        z3[:, sl], zr[:], SLOPE, zr[:],
                        Alu.mult, Alu.max, accum_out=ssum3[:, i:i + 1])
                    scr = ch.tile([C, CH23], dt, tag="scr2")
                    nc.scalar.activation(scr[:], z3[:, sl], Act.Square,
                                         accum_out=qsum3[:, i:i + 1])

                sc3, bi3 = stats_and_scale(3, ssum3, qsum3, NCH23, NCH23,
                                           gbs[:, 4:5], gbs[:, 5:6])

                # BN3-apply + max-pool over K
                yslab = sp.tile([C, GPC], dt, tag="yslab")
                for i in range(NPL):
                    sl = slice(i * CHPL, (i + 1) * CHPL)
                    yt = ch.tile([C, CHPL], dt, tag="ybn")
                    nc.vector.tensor_scalar(yt[:], z3[:, sl],
                                            sc3[:], bi3[:], Alu.mult, Alu.add)
                    g0 = (i * CHPL) // K
                    ng = CHPL // K
                    nc.vector.tensor_reduce(
                        yslab[:, g0:g0 + ng],
                        yt[:].rearrange("p (g k) -> p g k", k=K),
                        mybir.AxisListType.X, Alu.max)
                nc.sync.dma_start(y[:], yslab[:])

    _split_multi_waits(nc)
    return nc


# ------------------------------------------------------- cached AOT executable
def _get_exec():
    if "exec" in _CACHE:
        return _CACHE["exec"]

    import jax
    from concourse import bass2jax
    import concourse.mybir as mybir
    from jax.experimental.shard_map import shard_map
    from jax.sharding import Mesh, PartitionSpec

    nc = _build_nc()
    bass2jax.install_neuronx_cc_hook()

    partition_name = (nc.partition_id_tensor.name
                      if nc.partition_id_tensor else None)
    in_names, out_names, out_avals, zero_shapes = [], [], [], []
    in_shapes = {}
    for alloc in nc.m.functions[0].allocations:
        if not isinstance(alloc, mybir.MemoryLocationSet):
            continue
        name = alloc.memorylocations[0].name
        if alloc.kind == "ExternalInput":
            if name != partition_name:
                in_names.append(name)
                in_shapes[name] = (tuple(alloc.tensor_shape),
                                   mybir.dt.np(alloc.dtype))
        elif alloc.kind == "ExternalOutput":
            out_names.append(name)
            shape = tuple(alloc.tensor_shape)
            dtype = mybir.dt.np(alloc.dtype)
            out_avals.append(jax.core.ShapedArray(shape, dtype))
            zero_shapes.append((shape, dtype))
    n_params = len(in_names)
    n_outs = len(out_avals)
    all_in_names = list(in_names) + list(out_names)
    if partition_name is not None:
        all_in_names.append(partition_name)

    def _body(*args):
        operands = list(args)
        if partition_name is not None:
            operands.append(bass2jax.partition_id_tensor())
        outs = bass2jax._bass_exec_p.bind(
            *operands,
            out_avals=tuple(out_avals),
            in_names=tuple(all_in_names),
            out_names=tuple(out_names),
            lowering_input_output_aliases=(),
            sim_require_finite=True,
            sim_require_nnan=True,
            nc=nc,
        )
        return tuple(outs)

    devices = jax.devices()[:N_CORES]
    mesh = Mesh(np.asarray(devices), ("core",))
    in_specs = (PartitionSpec("core"),) * (n_params + n_outs)
    out_specs = (PartitionSpec("core"),) * n_outs

    def compile_fn():
        jitted = jax.jit(
            shard_map(_body, mesh=mesh, in_specs=in_specs,
                      out_specs=out_specs, check_rep=False),
            keep_unused=True)
        arg_specs = [
            jax.ShapeDtypeStruct((N_CORES * in_shapes[nm][0][0],
                                  *in_shapes[nm][0][1:]), in_shapes[nm][1])
            for nm in in_names
        ] + [
            jax.ShapeDtypeStruct((N_CORES * s[0], *s[1:]), d)
            for (s, d) in zero_shapes
        ]
        return jitted.lower(*arg_specs).compile()

    compiled = bass2jax.fast_dispatch_compile(compile_fn)
    # undonated zero output buffers: shipped to device once, reused per call
    from jax.sharding import NamedSharding
    shardings = [NamedSharding(mesh, PartitionSpec("core"))] * n_outs
    dev_zeros = [
        jax.device_put(np.zeros((N_CORES * s[0], *s[1:]), d), sh)
        for (s, d), sh in zip(zero_shapes, shardings)
    ]
    _CACHE["exec"] = (compiled, in_names, zero_shapes, dev_zeros)
    return _CACHE["exec"]


def kernel(p, W1, g1, b1, W2, g2, b2, W3, g3, b3):
    p = np.ascontiguousarray(np.asarray(p, np.float32))
    px = np.ascontiguousarray(p[:, :, 0])
    py = np.ascontiguousarray(p[:, :, 1])
    pz = np.ascontiguousarray(p[:, :, 2])
    idx = _fps(px, py, pz)                               # [B, M] int32
    p1 = np.take_along_axis(p, idx[:, :, None].astype(np.int64), axis=1)

    W1 = np.asarray(W1, np.float32)
    Ut = np.ascontiguousarray((W1[:, 0:3] + W1[:, 3:6]).T)   # [3, C]
    Vt = np.ascontiguousarray(W1[:, 0:3].T)                  # [3, C]
    uv = np.concatenate([Ut, Vt], axis=1)                    # [3, 2C]
    w2t = np.ascontiguousarray(np.asarray(W2, np.float32).T)
    w3t = np.ascontiguousarray(np.asarray(W3, np.float32).T)
    gbm = np.stack([g1, b1, g2, b2, g3, b3], axis=1).astype(np.float32)

    compiled, in_names, zero_shapes, dev_zeros = _get_exec()

    per_core = {"p5": [], "q5": [], "p1t": []}
    for c in range(N_CORES):
        b = c // 2
        h = c % 2
        pb = p[b]                                            # [N, 3]
        p2 = (pb[:, 0] * pb[:, 0] + pb[:, 1] * pb[:, 1]
              + pb[:, 2] * pb[:, 2]).astype(np.float32)
        p5c = np.empty((5, N), np.float32)
        p5c[0:3] = pb.T
        p5c[3] = p2
        p5c[4] = 1.0
        q = p1[b, h * GPC:(h + 1) * GPC]                     # [GPC, 3]
        q2 = (q[:, 0] * q[:, 0] + q[:, 1] * q[:, 1]
              + q[:, 2] * q[:, 2]).astype(np.float32)
        q5c = np.empty((5, GPC), np.float32)
        q5c[0:3] = 2.0 * q.T
        q5c[3] = -1.0
        q5c[4] = -q2
        per_core["p5"].append(p5c)
        per_core["q5"].append(q5c)
        per_core["p1t"].append(np.ascontiguousarray(q.T))

    rep = {"uv": uv, "w2t": w2t, "w3t": w3t, "gb": gbm}
    concat_in = []
    for nm in in_names:
        if nm in per_core:
            concat_in.append(
                np.ascontiguousarray(np.concatenate(per_core[nm], axis=0)))
        else:
            concat_in.append(np.concatenate([rep[nm]] * N_CORES, axis=0))
    outs = compiled(*concat_in, *dev_zeros)
    Y = np.asarray(outs[0]).reshape(N_CORES, C, GPC)
    Yc = np.concatenate(list(Y), axis=1)                   # [64, 8192]
    out = Yc.reshape(C, B, M).transpose(1, 0, 2)           # [B, 64, M]
    return np.ascontiguousarray(out.astype(np.float32))
